# revision 9
# baseline (speedup 1.0000x reference)
"""DialogueGCN forward as a Bass/Tile kernel on 8 TRN2 NeuronCores.

Sharding: data-parallel over dialogues (batch). Each core owns 32 contiguous
dialogues; edges never cross dialogues so all graph aggregation is local.

Optimizations over the 451us starting point (now ~167us):
- band restriction: attention scores live in a +-10 window; scale/exp/Shat
  matmuls and vector ops process ~340 of 900 column-units per u-tile group,
  with per-column-piece PSUM start/stop handling for the tile overlaps.
- fp16 matmul inputs (1 cycle/row at any free size; fp32r needs >=256 cols).
- fp8 DoubleRow projection: G drained to fp8e4 (x8 scale) in ch-plane pairs,
  relation weights x32 fp8 on host; one DR matmul per (speaker,dir) contracts
  both 100-row d-chunks at 0.5 cycles/row (proj 4800c -> 1200c). root+bias_r
  (x256) accumulate into both tau PSUMs; h1f = Act(ph1 * 1/256) -> fp16.
- speaker select: host-precomputed predicate row resident in SBUF (fp16,
  int16-bitcast) + copy_predicated between the two tau PSUMs.
- logits computed transposed (n=6 matmuls), drained straight into the
  batched l_out tile; log-softmax runs in 2 column chunks inside the loop;
  output stays in SBUF-native layout, host re-layouts.
- DVE fast modes: only 2-operand ops get them (TT 2x, TSP/Copy 4x with
  2-byte dtypes); scalar_tensor_tensor with op1!=bypass or accum_out runs 1x,
  so sums use one win-masked accum stt and Shat/xr use TT/TSP.
- one fused [128,1200] input DMA per dialogue (HWDGE costs 625ns per
  instruction); masks/large consts ride SWDGE; tmb split into 8 chunks so
  its transfer never monopolizes the DMA-engine pool.
- software-pipelined emission (6-stage skew): per iteration the engine
  queues see [loads(i+1) | scale+exp(i) | select+h1f(i-3) | G-mms(i-1)
  interleaved with DR-proj(i-2) + drains | qt(i-3) | h2(i-4) | hid(i-5) |
  logits(i-6) | win-sums+Shat+xr(i) at the DVE tail].

PSUM (8 banks): 4-bank ring (scale, 8 G combos, qt, logitsT), 2 banks
ph1_0/ph1_1, 2-bank ph2/phid ring. Engines steady-state: Act ~86% busy
(exp, 5 G-drains, h1f, qt-drain, h2/hid acts), DVE ~80%, PE ~65%.
"""

import os

import numpy as np

import concourse.bass as bass
import concourse.mybir as mybir
import concourse.tile as tile
from concourse import bass_utils

SEQ, BATCH, D, H, NCLS = 300, 256, 200, 128, 6
WP = WF = 10
NCORES = 8
BPC = BATCH // NCORES  # dialogues per core

UT = [(0, 128), (128, 128), (256, 44)]   # seq tiles (offset, size)
C0 = [0, 118, 246]                       # band col start per u-tile
C1 = [138, 266, 300]                     # band col end per u-tile
BW = [138, 148, 54]                      # band widths
SCOFF = [0, 138, 286]                    # scale psum col offsets (340 tot)
BWMAX = 148
# column pieces for banded PSUM accumulation: (a, b, contributing u-tiles)
PIECES = [(0, 118, (0,)), (118, 138, (0, 1)), (138, 246, (1,)),
          (246, 266, (1, 2)), (266, 300, (2,))]

F32 = mybir.dt.float32
F32R = mybir.dt.float32r
F16 = mybir.dt.float16
F8 = mybir.dt.float8e4
I16 = mybir.dt.int16
W8SCALE = 32.0   # host scale on w8 (fp8 range)
GSCALE = 8.0     # on-chip scale on G drains (fp8 range)
H1SCALE = W8SCALE * GSCALE  # ph1 carries h1 * H1SCALE

_CACHE = {}


def _split_multiwaits(nc, max_waits=1):
    """walrus in this container rejects >1 sem wait on an instruction
    ("Too many sync wait commands"); hoist extras onto preceding NOPs."""
    n = 0
    for f in nc.m.functions:
        for b in f.blocks:
            newlist = []
            changed = False
            for ins in b.instructions:
                si = ins.sync_info
                if si is not None and si.on_wait is not None and len(si.on_wait) > max_waits:
                    waits = list(si.on_wait)
                    for w in waits[max_waits:]:
                        n += 1
                        nop = mybir.InstNoOp(name=f"waitsplit-{n}", ins=[], outs=[])
                        nop.engine = ins.engine
                        nop.sync_info = mybir.SyncInfo(on_wait=[w], on_update=[])
                        newlist.append(nop)
                        nc.inst_map[nop.name] = nop
                    ins.sync_info = mybir.SyncInfo(
                        on_wait=waits[:max_waits],
                        on_update=list(si.on_update) if si.on_update else [],
                    )
                    changed = True
                newlist.append(ins)
            if changed:
                b.instructions = newlist
    return n


def _build_program():
    nc = bass.Bass("TRN2", num_devices=NCORES)

    ap = {}
    def din(name, shape, dt=F32):
        ap[name] = nc.dram_tensor(name, shape, dt, kind="ExternalInput").ap()

    # per-dialogue inputs: x in both layouts fused into one [128, 1200] load
    # (cols 0:600 = x seq-major 3 u-tiles, cols 600:1200 = x^T 2 d-chunks)
    din("xz16", (BPC, 128, 2 * 3 * D), F16)
    din("mskc", (128, BPC * 6), F32)           # speaker one-hot cols (b,s,k)
    # constants
    din("d01", (2, 3, 128, BWMAX), F16)        # dir masks, band-local
    din("winb", (3, 128, BWMAX), F16)          # window mask, band-local
    din("watt16", (100, 2, SEQ), F16)          # W_att d-chunks
    din("w8q", (100, 2, 8, H), F8)             # relation weights x32, fp8
    din("brcrow", (1, H), F16)                 # bias_r x256 as a row
    din("onesrow", (1, SEQ), F16)
    din("rwl", (100, 2, 2, H), F16)            # root x256 | wlind, d-chunks
    din("www", (3, H, H), F16)                 # w1, w2, wlinh
    din("wfc16", (H, NCLS), F16)
    din("biases", (128, 4), F32)               # brc, bgc, blc, spare
    din("bfcb", (128, BPC * 3 * NCLS), F32)    # b_fc broadcast over l_out
    din("tmb16", (128, BPC * SEQ), F16)        # target-speaker-0 predicate
    # output in SBUF-native layout [128, b*18 + k*6 + c]; host re-layouts
    out = nc.dram_tensor("out", (128, BPC * 3 * NCLS), F32,
                         kind="ExternalOutput").ap()

    repeat = int(os.environ.get("BASS_REPEAT", "1"))
    from contextlib import ExitStack
    with tile.TileContext(nc) as tc:
        with ExitStack() as ctx:
            pools = _mk_pools(tc, ctx)
            if repeat > 1:
                with tc.For_i(0, repeat, 1):
                    _body(nc, tc, ap, out, pools)
            else:
                _body(nc, tc, ap, out, pools)

    _split_multiwaits(nc)
    return nc


def _mk_pools(tc, ctx):
    return dict(
        cpool=ctx.enter_context(tc.tile_pool(name="const", bufs=1)),
        io=ctx.enter_context(tc.tile_pool(name="io", bufs=2)),
        wk=ctx.enter_context(tc.tile_pool(name="wk", bufs=3)),
        gpool=ctx.enter_context(tc.tile_pool(name="gpool", bufs=3)),
        ps_gb=ctx.enter_context(tc.tile_pool(name="ps_gb", bufs=4, space="PSUM")),
        ps_h1=ctx.enter_context(tc.tile_pool(name="ps_h1", bufs=1, space="PSUM")),
        ps_big=ctx.enter_context(tc.tile_pool(name="ps_big", bufs=2, space="PSUM")),
    )


AF = mybir.ActivationFunctionType
OP = mybir.AluOpType


class _Consts:
    pass


def _load_consts(nc, cpool, ap, c=None, phase=0):
    if c is None:
        c = _Consts()
    def ctile(name, shape, dt=F32):
        t = cpool.tile(shape, dt, name=f"c_{name}")
        setattr(c, name, t)
        return t

    if phase == 0:
        # needed for the very first scale matmuls
        t = ctile("mskc", [128, BPC * 6], F32)
        nc.gpsimd.dma_start(t[:], ap["mskc"])
        t = ctile("watt", [100, 2 * SEQ], F16)
        nc.sync.dma_start(t.rearrange("p (c t) -> p c t", c=2), ap["watt16"])
        return c
    if phase == 1:
        # needed by the first iteration's vector stage
        t = ctile("winb", [128, 3 * BWMAX], F16)
        nc.sync.dma_start(t.rearrange("p (k w) -> p k w", k=3),
                          ap["winb"].transpose([1, 0, 2]))
        t = ctile("d01", [128, 2 * 3 * BWMAX], F16)
        nc.sync.dma_start(t.rearrange("p (d k w) -> p d k w", d=2, k=3),
                          ap["d01"].transpose([2, 0, 1, 3]))
        return c
    # HWDGE: weights needed in iterations 2-4 (per-dialogue loads share it)
    t = ctile("www", [H, 3 * H], F16)
    nc.sync.dma_start(t.rearrange("p (j h) -> p j h", j=3),
                      ap["www"].transpose([1, 0, 2]))
    t = ctile("w8", [100, 2 * 8 * H], F8)
    nc.sync.dma_start(t.rearrange("p (c r h) -> p c r h", c=2, r=8), ap["w8q"])
    t = ctile("rwl", [100, 4 * H], F16)
    nc.sync.dma_start(t.rearrange("p (c j h) -> p c j h", c=2, j=2), ap["rwl"])
    # SWDGE (Pool queue, idle in prologue): late-need / large constants.
    # tmb is DMA'd in 4 chunks from the main loop so its 6.8us transfer
    # doesn't monopolize the DMA-engine pool during the prologue.
    ctile("tmb", [128, BPC * SEQ], F16)
    t = ctile("brcrow", [1, H], F16)
    nc.gpsimd.dma_start(t[:], ap["brcrow"])
    t = ctile("onesrow", [1, SEQ], F16)
    nc.gpsimd.dma_start(t[:], ap["onesrow"])
    t = ctile("biases", [128, 4], F32)
    nc.gpsimd.dma_start(t[:], ap["biases"])
    t = ctile("wfc", [H, NCLS], F16)
    nc.gpsimd.dma_start(t[:], ap["wfc16"])
    t = ctile("bfcb", [128, BPC * 3 * NCLS], F32)
    nc.gpsimd.dma_start(t[:], ap["bfcb"])
    c.l_out = cpool.tile([128, BPC * 3 * NCLS], F32, name="c_lout")

    # slice helpers
    c.watt_s = lambda ch, u0, uk: c.watt[:, ch * SEQ + u0: ch * SEQ + u0 + uk]
    c.dir_s = lambda dd, k, w: c.d01[:, (dd * 3 + k) * BWMAX: (dd * 3 + k) * BWMAX + w]
    c.win_s = lambda k, w: c.winb[:, k * BWMAX: k * BWMAX + w]
    # DoubleRow lhsT for relation r: [100, 2(ch-planes), H]
    c.w8_dr = lambda r: c.w8.rearrange("p (c r h) -> p c r h", c=2, r=8)[:, :, r, :]
    c.root_s = lambda ch: c.rwl[:, (ch * 2 + 0) * H: (ch * 2 + 0 + 1) * H]
    c.wlind_s = lambda ch: c.rwl[:, (ch * 2 + 1) * H: (ch * 2 + 1 + 1) * H]
    c.w1 = c.www[:, 0 * H:1 * H]
    c.w2 = c.www[:, 1 * H:2 * H]
    c.wlinh = c.www[:, 2 * H:3 * H]
    c.brc = c.biases[:, 0:1]
    c.bgc = c.biases[:, 1:2]
    c.blc = c.biases[:, 2:3]
    return c


def _body(nc, tc, ap, out, pools):
    cpool, io, wk, gpool = pools["cpool"], pools["io"], pools["wk"], pools["gpool"]
    ps_gb, ps_h1, ps_big = pools["ps_gb"], pools["ps_h1"], pools["ps_big"]

    st = {}  # per-dialogue live tiles

    def gb_bank():
        return ps_gb.tile([128, 512], F32, name="gb", tag="gb")

    def loads(b):
        d = st.setdefault(b, {})
        xz = io.tile([128, 2 * 3 * D], F16, name="xz", tag="xz", bufs=7)
        nc.sync.dma_start(xz[:], ap["xz16"][b])
        d["xz"] = d["xn"] = xz  # xn view: cols [0:600) = [k, d] blocks
        d["mc"] = c.mskc[:, b * 6:(b + 1) * 6]

    def xt_s(b, ch, a, w):  # xt [d-chunk, t] slice (cols 600:1200 of xz)
        return st[b]["xz"][:100, 600 + ch * SEQ + a: 600 + ch * SEQ + a + w]

    def s1a(b, c):
        """scale matmuls + one merged exp + win-masked row sums + rm. PE/Act/DVE."""
        d = st[b]
        psc = gb_bank()
        for k, (u0, uk) in enumerate(UT):
            o = SCOFF[k]
            for ch in range(2):
                nc.tensor.matmul(psc[:uk, o:o + BW[k]], c.watt_s(ch, u0, uk),
                                 xt_s(b, ch, C0[k], BW[k]),
                                 start=(ch == 0), stop=(ch == 1))
        p = wk.tile([128, 340], F16, name="p", tag="p")
        nc.scalar.activation(p[:, :], psc[:, :340], AF.Exp)
        d["p"] = p
        d["sm3"] = wk.tile([128, 3], F32, name="sm3", tag="sm3")
        d["rc3"] = wk.tile([128, 3], F32, name="rc3", tag="rc3")

    def s1b(b, c):
        """win-masked row sums -> rm, then Shat'_{s,dd} = (P*mask_s/sums)*dir_dd.
        All DVE, emitted at iteration tail (consumed by G-mms next iter)."""
        d = st[b]
        p = d["p"]
        sm3, rc3 = d["sm3"], d["rc3"]
        d["pw"] = []
        d["rm"] = {}
        for k, (u0, uk) in enumerate(UT):
            pw = wk.tile([128, BWMAX], F16, name=f"pw{k}", tag=f"pw{k}")
            nc.vector.scalar_tensor_tensor(
                pw[:uk, :BW[k]], p[:uk, SCOFF[k]:SCOFF[k] + BW[k]], 1.0,
                c.win_s(k, BW[k])[:uk, :],
                op0=OP.mult, op1=OP.mult,
                accum_out=sm3[:uk, k:k + 1],
            )
            d["pw"].append(pw)
        nc.vector.reciprocal(rc3[:, :], sm3[:, :])
        for k, (u0, uk) in enumerate(UT):
            rm2 = wk.tile([128, 2], F32, name=f"rm{k}", tag=f"rm{k}")
            nc.vector.tensor_scalar_mul(rm2[:uk, :], d["mc"][:uk, k::3],
                                        rc3[:uk, k:k + 1])
            for s in range(2):
                d["rm"][(s, k)] = rm2[:, s:s + 1]
        d["shat"] = {}
        d["xr"] = []
        for k, (u0, uk) in enumerate(UT):
            for dd in range(2):
                stt = wk.tile([128, BWMAX], F16, name=f"st{dd}{k}",
                              tag=f"st{dd}{k}")
                nc.vector.tensor_tensor(
                    stt[:uk, :BW[k]], d["pw"][k][:uk, :BW[k]],
                    c.dir_s(dd, k, BW[k])[:uk, :], op=OP.mult)
                d["shat"][(dd, k)] = stt
            xr = wk.tile([128, 2 * D], F16, name=f"xr{k}", tag=f"xr{k}")
            for s in range(2):
                nc.vector.tensor_scalar_mul(
                    xr[:uk, s * D:(s + 1) * D],
                    d["xn"][:uk, k * D:(k + 1) * D], d["rm"][(s, k)][:uk, :])
            d["xr"].append(xr)

    COMBOS = [(s, dd, ch) for s in range(2) for dd in range(2) for ch in range(2)]

    def s23(bg, bp, c):
        """G banded matmuls + fp8 drains for dialogue bg, interleaved with
        DoubleRow projection + root/bias matmuls for dialogue bp (keeps PE
        fed while Act/DVE chase the PSUM drains)."""
        proj = []  # (tau, idx, s, dd) DoubleRow matmul argument list
        if bp >= 0:
            for tau in range(2):
                for i, (s, dd) in enumerate([(0, 0), (0, 1), (1, 0), (1, 1)]):
                    proj.append((tau, i, s, dd))
        pi = 0
        ph1 = {}
        if bp >= 0:
            for tau in range(2):
                ph1[tau] = ps_h1.tile([128, SEQ], F32, name=f"h1_{tau}",
                                      tag=f"h1_{tau}")
            st[bp]["ph1"] = ph1

        # delay proj until combo 3 so ph1 banks are freed (select/h1f of the
        # older dialogue run at DVE/Act queue fronts) before the first DR mm
        PROJ_SCHED = [0, 0, 0, 2, 2, 2, 1, 1]

        def emit_proj(n):
            nonlocal pi
            for _ in range(n):
                if pi >= len(proj):
                    return
                tau, i, s, dd = proj[pi]
                r = s * 4 + tau * 2 + dd
                g = st[bp]["g"][(s, dd)]
                nc.tensor.matmul(ph1[tau][:, :], c.w8_dr(r),
                                 g.rearrange("p (c t) -> p c t", c=2)[:100, :, :],
                                 start=(i == 0), stop=False,
                                 perf_mode=mybir.MatmulPerfMode.DoubleRow)
                pi += 1
                if pi == len(proj):
                    # root (x256) + bias_r (x256) accumulated into both taus
                    for tau2 in range(2):
                        for ch in range(2):
                            nc.tensor.matmul(ph1[tau2][:, :], c.root_s(ch),
                                             xt_s(bp, ch, 0, SEQ),
                                             start=False, stop=False)
                        nc.tensor.matmul(ph1[tau2][:, :], c.brcrow[:1, :],
                                         c.onesrow[:1, :],
                                         start=False, stop=True)

        if bg >= 0:
            d = st[bg]
            d["g"] = {}
            gi = 0
            for (s, dd, ch) in COMBOS:
                pg = gb_bank()
                for (a, e, ks) in PIECES:
                    for j, k in enumerate(ks):
                        u0, uk = UT[k]
                        nc.tensor.matmul(
                            pg[:100, a:e],
                            d["xr"][k][:uk, s * D + ch * 100: s * D + ch * 100 + 100],
                            d["shat"][(dd, k)][:uk, a - C0[k]:e - C0[k]],
                            start=(j == 0), stop=(j == len(ks) - 1))
                if ch == 0:
                    g = gpool.tile([128, 2 * SEQ], F8, name=f"g{s}{dd}",
                                   tag=f"g{s}{dd}")
                    d["g"][(s, dd)] = g
                else:
                    g = d["g"][(s, dd)]
                if gi not in (1, 3, 5):
                    nc.scalar.activation(g[:100, ch * SEQ:(ch + 1) * SEQ],
                                         pg[:100, :SEQ], AF.Identity, scale=GSCALE)
                else:
                    nc.vector.tensor_scalar_mul(g[:100, ch * SEQ:(ch + 1) * SEQ],
                                                pg[:100, :SEQ], GSCALE)
                emit_proj(PROJ_SCHED[gi])
                gi += 1
        emit_proj(len(proj))

    def s3bv(b, c):
        """target-speaker select + h1f = sel + bias_r + root-part. DVE."""
        d = st[b]
        ph1 = d["ph1"]
        nc.vector.copy_predicated(
            ph1[1][:, :],
            c.tmb.bitcast(I16)[:, b * SEQ:(b + 1) * SEQ],
            ph1[0][:, :])
        h1f = gpool.tile([H, SEQ], F16, name="h1f", tag="h1f")
        nc.scalar.activation(h1f[:], ph1[1][:, :], AF.Identity,
                             scale=1.0 / H1SCALE)
        d["h1f"] = h1f

    def s3bq(b, c):
        """qt = h1f^T W2 (3 u-tiles into one psum) + drain. PE/Act."""
        d = st[b]
        pq = gb_bank()
        for k, (u0, uk) in enumerate(UT):
            nc.tensor.matmul(pq[:uk, k * H:(k + 1) * H],
                             d["h1f"][:, u0:u0 + uk], c.w2,
                             start=True, stop=True)
        qt = gpool.tile([128, 3 * H], F16, name="qt", tag="qt")
        nc.scalar.copy(qt[:], pq[:, :3 * H])
        d["qt"] = qt

    def s3c(b, c):
        """h2 = W1^T h1f + banded qt aggregation + b_gc. PE/Act."""
        d = st[b]
        ph2 = ps_big.tile([H, SEQ], F32, name="big", tag="big")
        nc.tensor.matmul(ph2[:, :], c.w1, d["h1f"][:], start=True, stop=False)
        for (a, e, ks) in PIECES:
            for j, k in enumerate(ks):
                u0, uk = UT[k]
                nc.tensor.matmul(ph2[:, a:e], d["qt"][:uk, k * H:(k + 1) * H],
                                 c.win_s(k, BW[k])[:uk, a - C0[k]:e - C0[k]],
                                 start=False, stop=(j == len(ks) - 1))
        h2 = gpool.tile([H, SEQ], F16, name="h2", tag="h2")
        nc.scalar.activation(h2[:], ph2[:], AF.Identity, bias=c.bgc)
        d["h2"] = h2

    def s3d(b, c):
        """hid = relu(Wlin_d^T x + Wlin_h^T h2 + b_lin). PE/Act."""
        d = st[b]
        phid = ps_big.tile([H, SEQ], F32, name="big", tag="big")
        for ch in range(2):
            nc.tensor.matmul(phid[:, :], c.wlind_s(ch), xt_s(b, ch, 0, SEQ),
                             start=(ch == 0), stop=False)
        nc.tensor.matmul(phid[:, :], c.wlinh, d["h2"][:], start=False, stop=True)
        hid = gpool.tile([H, SEQ], F16, name="hid", tag="hid")
        nc.scalar.activation(hid[:], phid[:], AF.Relu, bias=c.blc)
        d["hid"] = hid

    def s3e(b, c):
        """logits^T per u-tile (n=6) + drain into l_out. PE/DVE."""
        d = st[b]
        plt = gb_bank()
        for k, (u0, uk) in enumerate(UT):
            nc.tensor.matmul(plt[:uk, k * NCLS:(k + 1) * NCLS],
                             d["hid"][:, u0:u0 + uk], c.wfc,
                             start=True, stop=True)
        nc.vector.tensor_copy(c.l_out[:, b * 18:(b + 1) * 18], plt[:, :18])
        del st[b]

    # ---- stage 2 (chunked): bias + log-softmax + output DMA per 4 dialogues
    NCH = 2
    CW = BPC // NCH * 3 * NCLS  # 144 cols per chunk
    s4t = {}

    def s4(cc):
        o = cc * CW
        gn = CW // NCLS  # 24 groups
        lb = s4t.setdefault("lb", cpool.tile([128, BPC * 3 * NCLS], F32,
                                             name="c_lb"))
        nc.vector.tensor_tensor(lb[:, o:o + CW], c.l_out[:, o:o + CW],
                                c.bfcb[:, o:o + CW], op=OP.add)
        l3 = lb[:, o:o + CW].rearrange("p (g c) -> p g c", c=NCLS)
        m9 = s4t.setdefault("m9", cpool.tile([128, BPC * 3], F32, name="c_m9"))
        m96 = m9[:, cc * gn:(cc + 1) * gn]
        nc.vector.reduce_max(m96, l3, axis=mybir.AxisListType.X)
        esb = s4t.setdefault("esb", cpool.tile([128, BPC * 3 * NCLS], F32,
                                               name="c_esb"))
        e3 = esb[:, o:o + CW].rearrange("p (g c) -> p g c", c=NCLS)
        nc.vector.tensor_tensor(e3, l3, m96.broadcast_to([128, gn, NCLS]),
                                op=OP.subtract)
        e2sb = s4t.setdefault("e2sb", cpool.tile([128, BPC * 3 * NCLS], F32,
                                                 name="c_e2sb"))
        nc.scalar.activation(e2sb[:, o:o + CW], esb[:, o:o + CW], AF.Exp)
        s9 = s4t.setdefault("s9", cpool.tile([128, BPC * 3], F32, name="c_s9"))
        s96 = s9[:, cc * gn:(cc + 1) * gn]
        nc.vector.reduce_sum(
            s96, e2sb[:, o:o + CW].rearrange("p (g c) -> p g c", c=NCLS),
            axis=mybir.AxisListType.X)
        lnz = s4t.setdefault("lnz", cpool.tile([128, BPC * 3], F32, name="c_lnz"))
        lnzc = lnz[:, cc * gn:(cc + 1) * gn]
        nc.scalar.activation(lnzc, s96, AF.Ln)
        lsm = s4t.setdefault("lsm", cpool.tile([128, BPC * 3], F32, name="c_lsm"))
        lsmc = lsm[:, cc * gn:(cc + 1) * gn]
        nc.vector.tensor_tensor(lsmc, m96, lnzc, op=OP.add)
        osb = s4t.setdefault("osb", cpool.tile([128, BPC * 3 * NCLS], F32,
                                               name="c_osb"))
        o3 = osb[:, o:o + CW].rearrange("p (g c) -> p g c", c=NCLS)
        nc.vector.tensor_tensor(o3, l3, lsmc.broadcast_to([128, gn, NCLS]),
                                op=OP.subtract)
        nc.sync.dma_start(out[:, o:o + CW], osb[:, o:o + CW])

    # ---- prologue + pipelined loop ----
    c = _load_consts(nc, cpool, ap, phase=0)
    loads(0)
    _load_consts(nc, cpool, ap, c=c, phase=1)
    loads(1)
    loads(2)
    _load_consts(nc, cpool, ap, c=c, phase=2)

    BPN = BPC // NCH
    TQ = BPC * SEQ // 8
    for i in range(BPC + 7):
        if 1 <= i < 9:
            j = i - 1
            nc.gpsimd.dma_start(c.tmb[:, j * TQ:(j + 1) * TQ],
                                ap["tmb16"][:, j * TQ:(j + 1) * TQ])
        if 3 <= i + 1 < BPC:
            loads(i + 1)
        if i < BPC:
            s1a(i, c)  # scale mms (PE front) + exp (Act front)
        if 0 <= i - 3 < BPC:
            s3bv(i - 3, c)  # select + h1f at DVE/Act queue fronts
        # G drains hit DVE/Act queues right after so the PSUM ring turns
        s23(i - 1 if i - 1 < BPC else -1, i - 2 if i - 2 < BPC else -1, c)
        if 0 <= i - 3 < BPC:
            s3bq(i - 3, c)
        if 0 <= i - 4 < BPC:
            s3c(i - 4, c)
        if 0 <= i - 5 < BPC:
            s3d(i - 5, c)
        if 0 <= i - 6 < BPC:
            s3e(i - 6, c)
        if (i - 7) % BPN == BPN - 1 and 0 <= i - 7 < BPC:
            s4((i - 7) // BPN)
        if i < BPC:
            s1b(i, c)  # win-stt/rm/dir-stts at DVE queue tail


def _host_prep(inputs):
    feats = np.asarray(inputs["features"], dtype=np.float32)    # (300,256,200)
    spk = np.asarray(inputs["speakers"])                        # (300,256)
    W_att = np.asarray(inputs["W_att"], dtype=np.float32)
    basis = np.asarray(inputs["basis"], dtype=np.float32)
    comp = np.asarray(inputs["comp"], dtype=np.float32)
    root = np.asarray(inputs["root"], dtype=np.float32)
    bias_r = np.asarray(inputs["bias_r"], dtype=np.float32)
    W1 = np.asarray(inputs["W1"], dtype=np.float32)
    W2 = np.asarray(inputs["W2"], dtype=np.float32)
    b_gc = np.asarray(inputs["b_gc"], dtype=np.float32)
    W_lin = np.asarray(inputs["W_lin"], dtype=np.float32)
    b_lin = np.asarray(inputs["b_lin"], dtype=np.float32)
    W_fc = np.asarray(inputs["W_fc"], dtype=np.float32)
    b_fc = np.asarray(inputs["b_fc"], dtype=np.float32)

    i = np.arange(SEQ)[:, None]
    j = np.arange(SEQ)[None, :]
    win = (j >= i - WP) & (j <= i + WF)
    dir0 = (win & (i < j)).astype(np.float32)
    dir1 = (win & (i >= j)).astype(np.float32)
    winm = win.astype(np.float32)

    # band-local masks, padded to (3, 128, BWMAX)
    def bandify(m):
        o = np.zeros((3, 128, BWMAX), np.float16)
        for k, (u0, uk) in enumerate(UT):
            o[k, :uk, :BW[k]] = m[u0:u0 + uk, C0[k]:C1[k]]
        return o

    d01 = np.stack([bandify(dir0), bandify(dir1)])              # (2,3,128,148)
    winb = bandify(winm)                                        # (3,128,148)

    import ml_dtypes
    w8 = np.einsum("rb,bdh->rdh", comp, basis).astype(np.float32)  # (8,200,128)
    w8q = np.ascontiguousarray(
        (w8 * W8SCALE).reshape(8, 2, 100, H).transpose(2, 1, 0, 3)
    ).astype(ml_dtypes.float8_e4m3fn)                           # (100,2,8,128)
    watt16 = np.ascontiguousarray(
        W_att.reshape(2, 100, SEQ).transpose(1, 0, 2)).astype(np.float16)
    rwl = np.stack([(root * H1SCALE).reshape(2, 100, H),
                    np.ascontiguousarray(W_lin[:D]).reshape(2, 100, H)],
                   axis=2).transpose(1, 0, 2, 3)                # (100,2,2,128)
    rwl = np.ascontiguousarray(rwl).astype(np.float16)
    brcrow = (bias_r * H1SCALE).astype(np.float16).reshape(1, H)
    onesrow = np.ones((1, SEQ), np.float16)
    www = np.stack([W1, W2, np.ascontiguousarray(W_lin[D:])]).astype(np.float16)
    biases = np.zeros((128, 4), np.float32)
    biases[:, 0] = bias_r
    biases[:, 1] = b_gc
    biases[:, 2] = b_lin
    bfcb = np.broadcast_to(np.tile(b_fc, BPC * 3)[None, :],
                           (128, BPC * 3 * NCLS)).astype(np.float32)
    bfcb = np.ascontiguousarray(bfcb)

    shared = {
        "d01": d01, "winb": winb,
        "watt16": watt16, "w8q": w8q, "brcrow": brcrow, "onesrow": onesrow,
        "rwl": rwl, "www": www,
        "wfc16": W_fc.astype(np.float16),
        "biases": biases, "bfcb": bfcb,
    }

    in_maps = []
    for cid in range(NCORES):
        bs = slice(cid * BPC, (cid + 1) * BPC)
        fb = feats[:, bs, :]                                    # (300,32,200)
        xz = np.zeros((BPC, 128, 2 * 3 * D), np.float16)
        xn = fb.transpose(1, 0, 2)                              # (32,300,200)
        for k, (u0, uk) in enumerate(UT):
            xz[:, :uk, k * D:(k + 1) * D] = xn[:, u0:u0 + uk, :]
        xt = fb.transpose(1, 2, 0)                              # (32,200,300)
        xz[:, :100, 600:900] = xt[:, :100, :]
        xz[:, :100, 900:1200] = xt[:, 100:, :]
        sp = spk[:, bs].T                                       # (32,300)
        mskc = np.zeros((128, BPC, 6), np.float32)
        for s in range(2):
            for k, (u0, uk) in enumerate(UT):
                mskc[:uk, :, s * 3 + k] = (sp[:, u0:u0 + uk] == s).T
        tmb16 = np.broadcast_to(
            (sp == 0).astype(np.float16).reshape(1, BPC * SEQ),
            (128, BPC * SEQ))
        m = {"xz16": xz, "mskc": np.ascontiguousarray(mskc.reshape(128, BPC * 6)),
             "tmb16": np.ascontiguousarray(tmb16)}
        m.update(shared)
        in_maps.append(m)
    return in_maps


def get_program():
    if "nc" not in _CACHE:
        _CACHE["nc"] = _build_program()
    return _CACHE["nc"]


def kernel(**inputs):
    nc = get_program()
    in_maps = _host_prep(inputs)
    res = bass_utils.run_bass_kernel_spmd(nc, in_maps, core_ids=list(range(NCORES)))
    outs = []
    for c in range(NCORES):
        arr = np.asarray(res.results[c]["out"])            # (128, 576)
        arr = arr.reshape(128, BPC, 3, NCLS).transpose(1, 2, 0, 3)
        outs.append(arr.reshape(BPC, 384, NCLS)[:, :SEQ, :].reshape(-1, NCLS))
    return np.concatenate(outs, axis=0)


# revision 10
# speedup vs baseline: 1.0005x; 1.0005x over previous
"""DialogueGCN forward as a Bass/Tile kernel on 8 TRN2 NeuronCores, v2.

Sharding: data-parallel over dialogues (batch). Each core owns 32 contiguous
dialogues; edges never cross dialogues so all graph aggregation is local.

v2 optimizations over the 451us baseline:
- fp16 matmul inputs everywhere except the projection (f32r, n=300):
  1 cycle/row at any free size (fp32r is 4x below 256 cols).
- band restriction: scores live in a 21-wide window; scale/exp/Shat/G
  matmuls and vector ops process ~340 of 900 column-units per group.
- consolidated DMAs (625ns HWDGE fixed cost each): 3 loads per dialogue.
- speaker-select predicate precomputed on host, resident in SBUF
  (kills the per-dialogue SWDGE broadcast).
- logits computed transposed (n=6 matmuls) - no PE transposes, no
  full-width logits drain.
- gpsimd (Pool) engine takes stt/sums/rm; Act/DVE split the PSUM drains.
- software-pipelined emission (~6-stage skew across dialogues) so each
  engine's in-order queue interleaves work of several dialogues.

PSUM (8 banks): shared 4-bank ring "gb" (roles: scale, 8 G combos, qt,
logits, proot), 2 banks ph1_0/ph1_1, 2 banks ph2/phid ring.
"""

import os

import numpy as np

import concourse.bass as bass
import concourse.mybir as mybir
import concourse.tile as tile
from concourse import bass_utils

SEQ, BATCH, D, H, NCLS = 300, 256, 200, 128, 6
WP = WF = 10
NCORES = 8
BPC = BATCH // NCORES  # dialogues per core

UT = [(0, 128), (128, 128), (256, 44)]   # seq tiles (offset, size)
C0 = [0, 118, 246]                       # band col start per u-tile
C1 = [138, 266, 300]                     # band col end per u-tile
BW = [138, 148, 54]                      # band widths
SCOFF = [0, 138, 286]                    # scale psum col offsets (340 tot)
BWMAX = 148
# column pieces for banded PSUM accumulation: (a, b, contributing u-tiles)
PIECES = [(0, 118, (0,)), (118, 138, (0, 1)), (138, 246, (1,)),
          (246, 266, (1, 2)), (266, 300, (2,))]

F32 = mybir.dt.float32
F32R = mybir.dt.float32r
F16 = mybir.dt.float16
F8 = mybir.dt.float8e4
I16 = mybir.dt.int16
W8SCALE = 32.0   # host scale on w8 (fp8 range)
GSCALE = 8.0     # on-chip scale on G drains (fp8 range)
H1SCALE = W8SCALE * GSCALE  # ph1 carries h1 * H1SCALE

_CACHE = {}


def _split_multiwaits(nc, max_waits=1):
    """walrus in this container rejects >1 sem wait on an instruction
    ("Too many sync wait commands"); hoist extras onto preceding NOPs."""
    n = 0
    for f in nc.m.functions:
        for b in f.blocks:
            newlist = []
            changed = False
            for ins in b.instructions:
                si = ins.sync_info
                if si is not None and si.on_wait is not None and len(si.on_wait) > max_waits:
                    waits = list(si.on_wait)
                    for w in waits[max_waits:]:
                        n += 1
                        nop = mybir.InstNoOp(name=f"waitsplit-{n}", ins=[], outs=[])
                        nop.engine = ins.engine
                        nop.sync_info = mybir.SyncInfo(on_wait=[w], on_update=[])
                        newlist.append(nop)
                        nc.inst_map[nop.name] = nop
                    ins.sync_info = mybir.SyncInfo(
                        on_wait=waits[:max_waits],
                        on_update=list(si.on_update) if si.on_update else [],
                    )
                    changed = True
                newlist.append(ins)
            if changed:
                b.instructions = newlist
    return n


def _build_program():
    nc = bass.Bass("TRN2", num_devices=NCORES)

    ap = {}
    def din(name, shape, dt=F32):
        ap[name] = nc.dram_tensor(name, shape, dt, kind="ExternalInput").ap()

    # per-dialogue inputs: x in both layouts fused into one [128, 1200] load
    # (cols 0:600 = x seq-major 3 u-tiles, cols 600:1200 = x^T 2 d-chunks)
    din("xz16", (BPC, 128, 2 * 3 * D), F16)
    din("mskc", (128, BPC * 6), F32)           # speaker one-hot cols (b,s,k)
    # constants
    din("d01", (2, 3, 128, BWMAX), F16)        # dir masks, band-local
    din("winb", (3, 128, BWMAX), F16)          # window mask, band-local
    din("watt16", (100, 2, SEQ), F16)          # W_att d-chunks
    din("w8q", (100, 2, 8, H), F8)             # relation weights x32, fp8
    din("brcrow", (1, H), F16)                 # bias_r x256 as a row
    din("onesrow", (1, SEQ), F16)
    din("rwl", (100, 2, 2, H), F16)            # root x256 | wlind, d-chunks
    din("www", (3, H, H), F16)                 # w1, w2, wlinh
    din("wfc16", (H, NCLS), F16)
    din("biases", (128, 4), F32)               # brc, bgc, blc, spare
    din("bfcb", (128, BPC * 3 * NCLS), F32)    # b_fc broadcast over l_out
    din("tmb16", (128, BPC * SEQ), F16)        # target-speaker-0 predicate
    # output in SBUF-native layout [128, b*18 + k*6 + c]; host re-layouts
    out = nc.dram_tensor("out", (128, BPC * 3 * NCLS), F32,
                         kind="ExternalOutput").ap()

    repeat = int(os.environ.get("BASS_REPEAT", "1"))
    from contextlib import ExitStack
    with tile.TileContext(nc) as tc:
        with ExitStack() as ctx:
            pools = _mk_pools(tc, ctx)
            if repeat > 1:
                with tc.For_i(0, repeat, 1):
                    _body(nc, tc, ap, out, pools)
            else:
                _body(nc, tc, ap, out, pools)

    _split_multiwaits(nc)
    return nc


def _mk_pools(tc, ctx):
    return dict(
        cpool=ctx.enter_context(tc.tile_pool(name="const", bufs=1)),
        io=ctx.enter_context(tc.tile_pool(name="io", bufs=2)),
        wk=ctx.enter_context(tc.tile_pool(name="wk", bufs=3)),
        gpool=ctx.enter_context(tc.tile_pool(name="gpool", bufs=3)),
        ps_gb=ctx.enter_context(tc.tile_pool(name="ps_gb", bufs=4, space="PSUM")),
        ps_h1=ctx.enter_context(tc.tile_pool(name="ps_h1", bufs=1, space="PSUM")),
        ps_big=ctx.enter_context(tc.tile_pool(name="ps_big", bufs=2, space="PSUM")),
    )


AF = mybir.ActivationFunctionType
OP = mybir.AluOpType


class _Consts:
    pass


def _load_consts(nc, cpool, ap, c=None, phase=0):
    if c is None:
        c = _Consts()
    def ctile(name, shape, dt=F32):
        t = cpool.tile(shape, dt, name=f"c_{name}")
        setattr(c, name, t)
        return t

    if phase == 0:
        # needed for the very first scale matmuls
        t = ctile("mskc", [128, BPC * 6], F32)
        nc.gpsimd.dma_start(t[:], ap["mskc"])
        t = ctile("watt", [100, 2 * SEQ], F16)
        nc.sync.dma_start(t.rearrange("p (c t) -> p c t", c=2), ap["watt16"])
        return c
    if phase == 1:
        # needed by the first iteration's vector stage
        t = ctile("winb", [128, 3 * BWMAX], F16)
        nc.sync.dma_start(t.rearrange("p (k w) -> p k w", k=3),
                          ap["winb"].transpose([1, 0, 2]))
        t = ctile("d01", [128, 2 * 3 * BWMAX], F16)
        nc.sync.dma_start(t.rearrange("p (d k w) -> p d k w", d=2, k=3),
                          ap["d01"].transpose([2, 0, 1, 3]))
        return c
    # HWDGE: weights needed in iterations 2-4 (per-dialogue loads share it)
    t = ctile("www", [H, 3 * H], F16)
    nc.sync.dma_start(t.rearrange("p (j h) -> p j h", j=3),
                      ap["www"].transpose([1, 0, 2]))
    t = ctile("w8", [100, 2 * 8 * H], F8)
    nc.sync.dma_start(t.rearrange("p (c r h) -> p c r h", c=2, r=8), ap["w8q"])
    t = ctile("rwl", [100, 4 * H], F16)
    nc.sync.dma_start(t.rearrange("p (c j h) -> p c j h", c=2, j=2), ap["rwl"])
    # SWDGE (Pool queue, idle in prologue): late-need / large constants.
    # tmb is DMA'd in 4 chunks from the main loop so its 6.8us transfer
    # doesn't monopolize the DMA-engine pool during the prologue.
    ctile("tmb", [128, BPC * SEQ], F16)
    t = ctile("brcrow", [1, H], F16)
    nc.gpsimd.dma_start(t[:], ap["brcrow"])
    t = ctile("onesrow", [1, SEQ], F16)
    nc.gpsimd.dma_start(t[:], ap["onesrow"])
    t = ctile("biases", [128, 4], F32)
    nc.gpsimd.dma_start(t[:], ap["biases"])
    t = ctile("wfc", [H, NCLS], F16)
    nc.gpsimd.dma_start(t[:], ap["wfc16"])
    t = ctile("bfcb", [128, BPC * 3 * NCLS], F32)
    nc.gpsimd.dma_start(t[:], ap["bfcb"])
    c.l_out = cpool.tile([128, BPC * 3 * NCLS], F32, name="c_lout")

    # slice helpers
    c.watt_s = lambda ch, u0, uk: c.watt[:, ch * SEQ + u0: ch * SEQ + u0 + uk]
    c.dir_s = lambda dd, k, w: c.d01[:, (dd * 3 + k) * BWMAX: (dd * 3 + k) * BWMAX + w]
    c.win_s = lambda k, w: c.winb[:, k * BWMAX: k * BWMAX + w]
    # DoubleRow lhsT for relation r: [100, 2(ch-planes), H]
    c.w8_dr = lambda r: c.w8.rearrange("p (c r h) -> p c r h", c=2, r=8)[:, :, r, :]
    c.root_s = lambda ch: c.rwl[:, (ch * 2 + 0) * H: (ch * 2 + 0 + 1) * H]
    c.wlind_s = lambda ch: c.rwl[:, (ch * 2 + 1) * H: (ch * 2 + 1 + 1) * H]
    c.w1 = c.www[:, 0 * H:1 * H]
    c.w2 = c.www[:, 1 * H:2 * H]
    c.wlinh = c.www[:, 2 * H:3 * H]
    c.brc = c.biases[:, 0:1]
    c.bgc = c.biases[:, 1:2]
    c.blc = c.biases[:, 2:3]
    return c


def _body(nc, tc, ap, out, pools):
    cpool, io, wk, gpool = pools["cpool"], pools["io"], pools["wk"], pools["gpool"]
    ps_gb, ps_h1, ps_big = pools["ps_gb"], pools["ps_h1"], pools["ps_big"]

    st = {}  # per-dialogue live tiles

    def gb_bank():
        return ps_gb.tile([128, 512], F32, name="gb", tag="gb")

    def loads(b):
        d = st.setdefault(b, {})
        xz = io.tile([128, 2 * 3 * D], F16, name="xz", tag="xz", bufs=9)
        nc.sync.dma_start(xz[:], ap["xz16"][b])
        d["xz"] = d["xn"] = xz  # xn view: cols [0:600) = [k, d] blocks
        d["mc"] = c.mskc[:, b * 6:(b + 1) * 6]

    def xt_s(b, ch, a, w):  # xt [d-chunk, t] slice (cols 600:1200 of xz)
        return st[b]["xz"][:100, 600 + ch * SEQ + a: 600 + ch * SEQ + a + w]

    def s1a(b, c):
        """scale matmuls + one merged exp + win-masked row sums + rm. PE/Act/DVE."""
        d = st[b]
        psc = gb_bank()
        for k, (u0, uk) in enumerate(UT):
            o = SCOFF[k]
            for ch in range(2):
                nc.tensor.matmul(psc[:uk, o:o + BW[k]], c.watt_s(ch, u0, uk),
                                 xt_s(b, ch, C0[k], BW[k]),
                                 start=(ch == 0), stop=(ch == 1))
        p = wk.tile([128, 340], F16, name="p", tag="p")
        nc.scalar.activation(p[:, :], psc[:, :340], AF.Exp)
        d["p"] = p
        d["sm3"] = wk.tile([128, 3], F32, name="sm3", tag="sm3")
        d["rc3"] = wk.tile([128, 3], F32, name="rc3", tag="rc3")

    def s1b(b, c):
        """win-masked row sums -> rm, then Shat'_{s,dd} = (P*mask_s/sums)*dir_dd.
        All DVE, emitted at iteration tail (consumed by G-mms next iter)."""
        d = st[b]
        p = d["p"]
        sm3, rc3 = d["sm3"], d["rc3"]
        d["pw"] = []
        d["rm"] = {}
        for k, (u0, uk) in enumerate(UT):
            pw = wk.tile([128, BWMAX], F16, name=f"pw{k}", tag=f"pw{k}")
            nc.vector.scalar_tensor_tensor(
                pw[:uk, :BW[k]], p[:uk, SCOFF[k]:SCOFF[k] + BW[k]], 1.0,
                c.win_s(k, BW[k])[:uk, :],
                op0=OP.mult, op1=OP.mult,
                accum_out=sm3[:uk, k:k + 1],
            )
            d["pw"].append(pw)
        nc.vector.reciprocal(rc3[:, :], sm3[:, :])
        for k, (u0, uk) in enumerate(UT):
            rm2 = wk.tile([128, 2], F32, name=f"rm{k}", tag=f"rm{k}")
            nc.vector.tensor_scalar_mul(rm2[:uk, :], d["mc"][:uk, k::3],
                                        rc3[:uk, k:k + 1])
            for s in range(2):
                d["rm"][(s, k)] = rm2[:, s:s + 1]
        d["shat"] = {}
        d["xr"] = []
        for k, (u0, uk) in enumerate(UT):
            for dd in range(2):
                stt = wk.tile([128, BWMAX], F16, name=f"st{dd}{k}",
                              tag=f"st{dd}{k}")
                nc.vector.tensor_tensor(
                    stt[:uk, :BW[k]], d["pw"][k][:uk, :BW[k]],
                    c.dir_s(dd, k, BW[k])[:uk, :], op=OP.mult)
                d["shat"][(dd, k)] = stt
            xr = wk.tile([128, 2 * D], F16, name=f"xr{k}", tag=f"xr{k}")
            for s in range(2):
                nc.vector.tensor_scalar_mul(
                    xr[:uk, s * D:(s + 1) * D],
                    d["xn"][:uk, k * D:(k + 1) * D], d["rm"][(s, k)][:uk, :])
            d["xr"].append(xr)

    COMBOS = [(s, dd, ch) for s in range(2) for dd in range(2) for ch in range(2)]

    def s23(bg, bp, c):
        """G banded matmuls + fp8 drains for dialogue bg, interleaved with
        DoubleRow projection + root/bias matmuls for dialogue bp (keeps PE
        fed while Act/DVE chase the PSUM drains)."""
        proj = []  # (tau, idx, s, dd) DoubleRow matmul argument list
        if bp >= 0:
            for tau in range(2):
                for i, (s, dd) in enumerate([(0, 0), (0, 1), (1, 0), (1, 1)]):
                    proj.append((tau, i, s, dd))
        pi = 0
        ph1 = {}
        if bp >= 0:
            for tau in range(2):
                ph1[tau] = ps_h1.tile([128, SEQ], F32, name=f"h1_{tau}",
                                      tag=f"h1_{tau}")
            st[bp]["ph1"] = ph1

        # delay proj until combo 3 so ph1 banks are freed (select/h1f of the
        # older dialogue run at DVE/Act queue fronts) before the first DR mm
        PROJ_SCHED = [0, 0, 0, 2, 2, 2, 1, 1]

        def emit_proj(n):
            nonlocal pi
            for _ in range(n):
                if pi >= len(proj):
                    return
                tau, i, s, dd = proj[pi]
                r = s * 4 + tau * 2 + dd
                g = st[bp]["g"][(s, dd)]
                nc.tensor.matmul(ph1[tau][:, :], c.w8_dr(r),
                                 g.rearrange("p (c t) -> p c t", c=2)[:100, :, :],
                                 start=(i == 0), stop=False,
                                 perf_mode=mybir.MatmulPerfMode.DoubleRow)
                pi += 1
                if pi == len(proj):
                    # root (x256) + bias_r (x256) accumulated into both taus
                    for tau2 in range(2):
                        for ch in range(2):
                            nc.tensor.matmul(ph1[tau2][:, :], c.root_s(ch),
                                             xt_s(bp, ch, 0, SEQ),
                                             start=False, stop=False)
                        nc.tensor.matmul(ph1[tau2][:, :], c.brcrow[:1, :],
                                         c.onesrow[:1, :],
                                         start=False, stop=True)

        if bg >= 0:
            d = st[bg]
            d["g"] = {}
            gi = 0
            for (s, dd, ch) in COMBOS:
                pg = gb_bank()
                for (a, e, ks) in PIECES:
                    for j, k in enumerate(ks):
                        u0, uk = UT[k]
                        nc.tensor.matmul(
                            pg[:100, a:e],
                            d["xr"][k][:uk, s * D + ch * 100: s * D + ch * 100 + 100],
                            d["shat"][(dd, k)][:uk, a - C0[k]:e - C0[k]],
                            start=(j == 0), stop=(j == len(ks) - 1))
                if ch == 0:
                    g = gpool.tile([128, 2 * SEQ], F8, name=f"g{s}{dd}",
                                   tag=f"g{s}{dd}")
                    d["g"][(s, dd)] = g
                else:
                    g = d["g"][(s, dd)]
                if gi not in (1, 3, 5):
                    nc.scalar.activation(g[:100, ch * SEQ:(ch + 1) * SEQ],
                                         pg[:100, :SEQ], AF.Identity, scale=GSCALE)
                else:
                    nc.vector.tensor_scalar_mul(g[:100, ch * SEQ:(ch + 1) * SEQ],
                                                pg[:100, :SEQ], GSCALE)
                emit_proj(PROJ_SCHED[gi])
                gi += 1
        emit_proj(len(proj))

    def s3bv(b, c):
        """target-speaker select + h1f = sel + bias_r + root-part. DVE."""
        d = st[b]
        ph1 = d["ph1"]
        nc.vector.copy_predicated(
            ph1[1][:, :],
            c.tmb.bitcast(I16)[:, b * SEQ:(b + 1) * SEQ],
            ph1[0][:, :])
        h1f = gpool.tile([H, SEQ], F16, name="h1f", tag="h1f")
        nc.scalar.activation(h1f[:], ph1[1][:, :], AF.Identity,
                             scale=1.0 / H1SCALE)
        d["h1f"] = h1f

    def s3bq(b, c):
        """qt = h1f^T W2 (3 u-tiles into one psum) + drain. PE/Act."""
        d = st[b]
        pq = gb_bank()
        for k, (u0, uk) in enumerate(UT):
            nc.tensor.matmul(pq[:uk, k * H:(k + 1) * H],
                             d["h1f"][:, u0:u0 + uk], c.w2,
                             start=True, stop=True)
        qt = gpool.tile([128, 3 * H], F16, name="qt", tag="qt")
        nc.scalar.copy(qt[:], pq[:, :3 * H])
        d["qt"] = qt

    def s3c(b, c):
        """h2 = W1^T h1f + banded qt aggregation + b_gc. PE/Act."""
        d = st[b]
        ph2 = ps_big.tile([H, SEQ], F32, name="big", tag="big")
        nc.tensor.matmul(ph2[:, :], c.w1, d["h1f"][:], start=True, stop=False)
        for (a, e, ks) in PIECES:
            for j, k in enumerate(ks):
                u0, uk = UT[k]
                nc.tensor.matmul(ph2[:, a:e], d["qt"][:uk, k * H:(k + 1) * H],
                                 c.win_s(k, BW[k])[:uk, a - C0[k]:e - C0[k]],
                                 start=False, stop=(j == len(ks) - 1))
        h2 = gpool.tile([H, SEQ], F16, name="h2", tag="h2")
        nc.scalar.activation(h2[:], ph2[:], AF.Identity, bias=c.bgc)
        d["h2"] = h2

    def s3d(b, c):
        """hid = relu(Wlin_d^T x + Wlin_h^T h2 + b_lin). PE/Act."""
        d = st[b]
        phid = ps_big.tile([H, SEQ], F32, name="big", tag="big")
        for ch in range(2):
            nc.tensor.matmul(phid[:, :], c.wlind_s(ch), xt_s(b, ch, 0, SEQ),
                             start=(ch == 0), stop=False)
        nc.tensor.matmul(phid[:, :], c.wlinh, d["h2"][:], start=False, stop=True)
        hid = gpool.tile([H, SEQ], F16, name="hid", tag="hid")
        nc.scalar.activation(hid[:], phid[:], AF.Relu, bias=c.blc)
        d["hid"] = hid

    def s3e(b, c):
        """logits^T per u-tile (n=6) + drain into l_out. PE/DVE."""
        d = st[b]
        plt = gb_bank()
        for k, (u0, uk) in enumerate(UT):
            nc.tensor.matmul(plt[:uk, k * NCLS:(k + 1) * NCLS],
                             d["hid"][:, u0:u0 + uk], c.wfc,
                             start=True, stop=True)
        nc.vector.tensor_copy(c.l_out[:, b * 18:(b + 1) * 18], plt[:, :18])
        del st[b]

    # ---- stage 2 (chunked): bias + log-softmax + output DMA per 4 dialogues
    NCH = 2
    CW = BPC // NCH * 3 * NCLS  # 144 cols per chunk
    s4t = {}

    def s4(cc):
        o = cc * CW
        gn = CW // NCLS  # 24 groups
        lb = s4t.setdefault("lb", cpool.tile([128, BPC * 3 * NCLS], F32,
                                             name="c_lb"))
        nc.vector.tensor_tensor(lb[:, o:o + CW], c.l_out[:, o:o + CW],
                                c.bfcb[:, o:o + CW], op=OP.add)
        l3 = lb[:, o:o + CW].rearrange("p (g c) -> p g c", c=NCLS)
        m9 = s4t.setdefault("m9", cpool.tile([128, BPC * 3], F32, name="c_m9"))
        m96 = m9[:, cc * gn:(cc + 1) * gn]
        nc.vector.reduce_max(m96, l3, axis=mybir.AxisListType.X)
        esb = s4t.setdefault("esb", cpool.tile([128, BPC * 3 * NCLS], F32,
                                               name="c_esb"))
        e3 = esb[:, o:o + CW].rearrange("p (g c) -> p g c", c=NCLS)
        nc.vector.tensor_tensor(e3, l3, m96.broadcast_to([128, gn, NCLS]),
                                op=OP.subtract)
        e2sb = s4t.setdefault("e2sb", cpool.tile([128, BPC * 3 * NCLS], F32,
                                                 name="c_e2sb"))
        nc.scalar.activation(e2sb[:, o:o + CW], esb[:, o:o + CW], AF.Exp)
        s9 = s4t.setdefault("s9", cpool.tile([128, BPC * 3], F32, name="c_s9"))
        s96 = s9[:, cc * gn:(cc + 1) * gn]
        nc.vector.reduce_sum(
            s96, e2sb[:, o:o + CW].rearrange("p (g c) -> p g c", c=NCLS),
            axis=mybir.AxisListType.X)
        lnz = s4t.setdefault("lnz", cpool.tile([128, BPC * 3], F32, name="c_lnz"))
        lnzc = lnz[:, cc * gn:(cc + 1) * gn]
        nc.scalar.activation(lnzc, s96, AF.Ln)
        lsm = s4t.setdefault("lsm", cpool.tile([128, BPC * 3], F32, name="c_lsm"))
        lsmc = lsm[:, cc * gn:(cc + 1) * gn]
        nc.vector.tensor_tensor(lsmc, m96, lnzc, op=OP.add)
        osb = s4t.setdefault("osb", cpool.tile([128, BPC * 3 * NCLS], F32,
                                               name="c_osb"))
        o3 = osb[:, o:o + CW].rearrange("p (g c) -> p g c", c=NCLS)
        nc.vector.tensor_tensor(o3, l3, lsmc.broadcast_to([128, gn, NCLS]),
                                op=OP.subtract)
        nc.sync.dma_start(out[:, o:o + CW], osb[:, o:o + CW])

    # ---- prologue + pipelined loop ----
    c = _load_consts(nc, cpool, ap, phase=0)
    loads(0)
    _load_consts(nc, cpool, ap, c=c, phase=1)
    loads(1)
    loads(2)
    _load_consts(nc, cpool, ap, c=c, phase=2)

    BPN = BPC // NCH
    TQ = BPC * SEQ // 8
    for i in range(BPC + 8):
        if 1 <= i < 9:
            j = i - 1
            nc.gpsimd.dma_start(c.tmb[:, j * TQ:(j + 1) * TQ],
                                ap["tmb16"][:, j * TQ:(j + 1) * TQ])
        if 3 <= i + 1 < BPC:
            loads(i + 1)
        if i < BPC:
            s1a(i, c)  # scale mms (PE front) + exp (Act front)
        if 0 <= i - 3 < BPC:
            s3bv(i - 3, c)  # select + h1f at DVE/Act queue fronts
        # G drains hit DVE/Act queues right after so the PSUM ring turns
        s23(i - 1 if i - 1 < BPC else -1, i - 2 if i - 2 < BPC else -1, c)
        if 0 <= i - 3 < BPC:
            s3bq(i - 3, c)
        if 0 <= i - 5 < BPC:
            s3c(i - 5, c)
        if 0 <= i - 6 < BPC:
            s3d(i - 6, c)
        if 0 <= i - 7 < BPC:
            s3e(i - 7, c)
        if (i - 8) % BPN == BPN - 1 and 0 <= i - 8 < BPC:
            s4((i - 8) // BPN)
        if i < BPC:
            s1b(i, c)  # win-stt/rm/dir-stts at DVE queue tail


def _host_prep(inputs):
    feats = np.asarray(inputs["features"], dtype=np.float32)    # (300,256,200)
    spk = np.asarray(inputs["speakers"])                        # (300,256)
    W_att = np.asarray(inputs["W_att"], dtype=np.float32)
    basis = np.asarray(inputs["basis"], dtype=np.float32)
    comp = np.asarray(inputs["comp"], dtype=np.float32)
    root = np.asarray(inputs["root"], dtype=np.float32)
    bias_r = np.asarray(inputs["bias_r"], dtype=np.float32)
    W1 = np.asarray(inputs["W1"], dtype=np.float32)
    W2 = np.asarray(inputs["W2"], dtype=np.float32)
    b_gc = np.asarray(inputs["b_gc"], dtype=np.float32)
    W_lin = np.asarray(inputs["W_lin"], dtype=np.float32)
    b_lin = np.asarray(inputs["b_lin"], dtype=np.float32)
    W_fc = np.asarray(inputs["W_fc"], dtype=np.float32)
    b_fc = np.asarray(inputs["b_fc"], dtype=np.float32)

    i = np.arange(SEQ)[:, None]
    j = np.arange(SEQ)[None, :]
    win = (j >= i - WP) & (j <= i + WF)
    dir0 = (win & (i < j)).astype(np.float32)
    dir1 = (win & (i >= j)).astype(np.float32)
    winm = win.astype(np.float32)

    # band-local masks, padded to (3, 128, BWMAX)
    def bandify(m):
        o = np.zeros((3, 128, BWMAX), np.float16)
        for k, (u0, uk) in enumerate(UT):
            o[k, :uk, :BW[k]] = m[u0:u0 + uk, C0[k]:C1[k]]
        return o

    d01 = np.stack([bandify(dir0), bandify(dir1)])              # (2,3,128,148)
    winb = bandify(winm)                                        # (3,128,148)

    import ml_dtypes
    w8 = np.einsum("rb,bdh->rdh", comp, basis).astype(np.float32)  # (8,200,128)
    w8q = np.ascontiguousarray(
        (w8 * W8SCALE).reshape(8, 2, 100, H).transpose(2, 1, 0, 3)
    ).astype(ml_dtypes.float8_e4m3fn)                           # (100,2,8,128)
    watt16 = np.ascontiguousarray(
        W_att.reshape(2, 100, SEQ).transpose(1, 0, 2)).astype(np.float16)
    rwl = np.stack([(root * H1SCALE).reshape(2, 100, H),
                    np.ascontiguousarray(W_lin[:D]).reshape(2, 100, H)],
                   axis=2).transpose(1, 0, 2, 3)                # (100,2,2,128)
    rwl = np.ascontiguousarray(rwl).astype(np.float16)
    brcrow = (bias_r * H1SCALE).astype(np.float16).reshape(1, H)
    onesrow = np.ones((1, SEQ), np.float16)
    www = np.stack([W1, W2, np.ascontiguousarray(W_lin[D:])]).astype(np.float16)
    biases = np.zeros((128, 4), np.float32)
    biases[:, 0] = bias_r
    biases[:, 1] = b_gc
    biases[:, 2] = b_lin
    bfcb = np.broadcast_to(np.tile(b_fc, BPC * 3)[None, :],
                           (128, BPC * 3 * NCLS)).astype(np.float32)
    bfcb = np.ascontiguousarray(bfcb)

    shared = {
        "d01": d01, "winb": winb,
        "watt16": watt16, "w8q": w8q, "brcrow": brcrow, "onesrow": onesrow,
        "rwl": rwl, "www": www,
        "wfc16": W_fc.astype(np.float16),
        "biases": biases, "bfcb": bfcb,
    }

    in_maps = []
    for cid in range(NCORES):
        bs = slice(cid * BPC, (cid + 1) * BPC)
        fb = feats[:, bs, :]                                    # (300,32,200)
        xz = np.zeros((BPC, 128, 2 * 3 * D), np.float16)
        xn = fb.transpose(1, 0, 2)                              # (32,300,200)
        for k, (u0, uk) in enumerate(UT):
            xz[:, :uk, k * D:(k + 1) * D] = xn[:, u0:u0 + uk, :]
        xt = fb.transpose(1, 2, 0)                              # (32,200,300)
        xz[:, :100, 600:900] = xt[:, :100, :]
        xz[:, :100, 900:1200] = xt[:, 100:, :]
        sp = spk[:, bs].T                                       # (32,300)
        mskc = np.zeros((128, BPC, 6), np.float32)
        for s in range(2):
            for k, (u0, uk) in enumerate(UT):
                mskc[:uk, :, s * 3 + k] = (sp[:, u0:u0 + uk] == s).T
        tmb16 = np.broadcast_to(
            (sp == 0).astype(np.float16).reshape(1, BPC * SEQ),
            (128, BPC * SEQ))
        m = {"xz16": xz, "mskc": np.ascontiguousarray(mskc.reshape(128, BPC * 6)),
             "tmb16": np.ascontiguousarray(tmb16)}
        m.update(shared)
        in_maps.append(m)
    return in_maps


def get_program():
    if "nc" not in _CACHE:
        _CACHE["nc"] = _build_program()
    return _CACHE["nc"]


def kernel(**inputs):
    nc = get_program()
    in_maps = _host_prep(inputs)
    res = bass_utils.run_bass_kernel_spmd(nc, in_maps, core_ids=list(range(NCORES)))
    outs = []
    for c in range(NCORES):
        arr = np.asarray(res.results[c]["out"])            # (128, 576)
        arr = arr.reshape(128, BPC, 3, NCLS).transpose(1, 2, 0, 3)
        outs.append(arr.reshape(BPC, 384, NCLS)[:, :SEQ, :].reshape(-1, NCLS))
    return np.concatenate(outs, axis=0)


# revision 11
# speedup vs baseline: 1.0009x; 1.0004x over previous
"""DialogueGCN forward as a Bass/Tile kernel on 8 TRN2 NeuronCores, v2.

Sharding: data-parallel over dialogues (batch). Each core owns 32 contiguous
dialogues; edges never cross dialogues so all graph aggregation is local.

v2 optimizations over the 451us baseline:
- fp16 matmul inputs everywhere except the projection (f32r, n=300):
  1 cycle/row at any free size (fp32r is 4x below 256 cols).
- band restriction: scores live in a 21-wide window; scale/exp/Shat/G
  matmuls and vector ops process ~340 of 900 column-units per group.
- consolidated DMAs (625ns HWDGE fixed cost each): 3 loads per dialogue.
- speaker-select predicate precomputed on host, resident in SBUF
  (kills the per-dialogue SWDGE broadcast).
- logits computed transposed (n=6 matmuls) - no PE transposes, no
  full-width logits drain.
- gpsimd (Pool) engine takes stt/sums/rm; Act/DVE split the PSUM drains.
- software-pipelined emission (~6-stage skew across dialogues) so each
  engine's in-order queue interleaves work of several dialogues.

PSUM (8 banks): shared 4-bank ring "gb" (roles: scale, 8 G combos, qt,
logits, proot), 2 banks ph1_0/ph1_1, 2 banks ph2/phid ring.
"""

import os

import numpy as np

import concourse.bass as bass
import concourse.mybir as mybir
import concourse.tile as tile
from concourse import bass_utils

SEQ, BATCH, D, H, NCLS = 300, 256, 200, 128, 6
WP = WF = 10
NCORES = 8
BPC = BATCH // NCORES  # dialogues per core

UT = [(0, 128), (128, 128), (256, 44)]   # seq tiles (offset, size)
C0 = [0, 118, 246]                       # band col start per u-tile
C1 = [138, 266, 300]                     # band col end per u-tile
BW = [138, 148, 54]                      # band widths
SCOFF = [0, 138, 286]                    # scale psum col offsets (340 tot)
BWMAX = 148
# column pieces for banded PSUM accumulation: (a, b, contributing u-tiles)
PIECES = [(0, 118, (0,)), (118, 138, (0, 1)), (138, 246, (1,)),
          (246, 266, (1, 2)), (266, 300, (2,))]

F32 = mybir.dt.float32
F32R = mybir.dt.float32r
F16 = mybir.dt.float16
F8 = mybir.dt.float8e4
I16 = mybir.dt.int16
W8SCALE = 32.0   # host scale on w8 (fp8 range)
GSCALE = 8.0     # on-chip scale on G drains (fp8 range)
H1SCALE = W8SCALE * GSCALE  # ph1 carries h1 * H1SCALE

_CACHE = {}


def _split_multiwaits(nc, max_waits=1):
    """walrus in this container rejects >1 sem wait on an instruction
    ("Too many sync wait commands"); hoist extras onto preceding NOPs."""
    n = 0
    for f in nc.m.functions:
        for b in f.blocks:
            newlist = []
            changed = False
            for ins in b.instructions:
                si = ins.sync_info
                if si is not None and si.on_wait is not None and len(si.on_wait) > max_waits:
                    waits = list(si.on_wait)
                    for w in waits[max_waits:]:
                        n += 1
                        nop = mybir.InstNoOp(name=f"waitsplit-{n}", ins=[], outs=[])
                        nop.engine = ins.engine
                        nop.sync_info = mybir.SyncInfo(on_wait=[w], on_update=[])
                        newlist.append(nop)
                        nc.inst_map[nop.name] = nop
                    ins.sync_info = mybir.SyncInfo(
                        on_wait=waits[:max_waits],
                        on_update=list(si.on_update) if si.on_update else [],
                    )
                    changed = True
                newlist.append(ins)
            if changed:
                b.instructions = newlist
    return n


def _build_program():
    nc = bass.Bass("TRN2", num_devices=NCORES)

    ap = {}
    def din(name, shape, dt=F32):
        ap[name] = nc.dram_tensor(name, shape, dt, kind="ExternalInput").ap()

    # per-dialogue inputs: x in both layouts fused into one [128, 1200] load
    # (cols 0:600 = x seq-major 3 u-tiles, cols 600:1200 = x^T 2 d-chunks)
    din("xz16", (BPC, 128, 2 * 3 * D), F16)
    din("mskc", (128, BPC * 6), F32)           # speaker one-hot cols (b,s,k)
    # constants
    din("d01", (2, 3, 128, BWMAX), F16)        # dir masks, band-local
    din("winb", (3, 128, BWMAX), F16)          # window mask, band-local
    din("watt16", (100, 2, SEQ), F16)          # W_att d-chunks
    din("w8q", (100, 2, 8, H), F8)             # relation weights x32, fp8
    din("brcrow", (1, H), F16)                 # bias_r x256 as a row
    din("onesrow", (1, SEQ), F16)
    din("rwl", (100, 2, 2, H), F16)            # root x256 | wlind, d-chunks
    din("www", (3, H, H), F16)                 # w1, w2, wlinh
    din("wfc16", (H, NCLS), F16)
    din("biases", (128, 4), F32)               # brc, bgc, blc, spare
    din("bfcb", (128, BPC * 3 * NCLS), F32)    # b_fc broadcast over l_out
    din("tmb16", (128, BPC * SEQ), F16)        # target-speaker-0 predicate
    # output in SBUF-native layout [128, b*18 + k*6 + c]; host re-layouts
    out = nc.dram_tensor("out", (128, BPC * 3 * NCLS), F32,
                         kind="ExternalOutput").ap()

    repeat = int(os.environ.get("BASS_REPEAT", "1"))
    from contextlib import ExitStack
    with tile.TileContext(nc) as tc:
        with ExitStack() as ctx:
            pools = _mk_pools(tc, ctx)
            if repeat > 1:
                with tc.For_i(0, repeat, 1):
                    _body(nc, tc, ap, out, pools)
            else:
                _body(nc, tc, ap, out, pools)

    _split_multiwaits(nc)
    return nc


def _mk_pools(tc, ctx):
    return dict(
        cpool=ctx.enter_context(tc.tile_pool(name="const", bufs=1)),
        io=ctx.enter_context(tc.tile_pool(name="io", bufs=2)),
        wk=ctx.enter_context(tc.tile_pool(name="wk", bufs=4)),
        gpool=ctx.enter_context(tc.tile_pool(name="gpool", bufs=4)),
        ps_gb=ctx.enter_context(tc.tile_pool(name="ps_gb", bufs=4, space="PSUM")),
        ps_h1=ctx.enter_context(tc.tile_pool(name="ps_h1", bufs=1, space="PSUM")),
        ps_big=ctx.enter_context(tc.tile_pool(name="ps_big", bufs=2, space="PSUM")),
    )


AF = mybir.ActivationFunctionType
OP = mybir.AluOpType


class _Consts:
    pass


def _load_consts(nc, cpool, ap, c=None, phase=0):
    if c is None:
        c = _Consts()
    def ctile(name, shape, dt=F32):
        t = cpool.tile(shape, dt, name=f"c_{name}")
        setattr(c, name, t)
        return t

    if phase == 0:
        # needed for the very first scale matmuls
        t = ctile("mskc", [128, BPC * 6], F32)
        nc.gpsimd.dma_start(t[:], ap["mskc"])
        t = ctile("watt", [100, 2 * SEQ], F16)
        nc.sync.dma_start(t.rearrange("p (c t) -> p c t", c=2), ap["watt16"])
        return c
    if phase == 1:
        # needed by the first iteration's vector stage
        t = ctile("winb", [128, 3 * BWMAX], F16)
        nc.sync.dma_start(t.rearrange("p (k w) -> p k w", k=3),
                          ap["winb"].transpose([1, 0, 2]))
        t = ctile("d01", [128, 2 * 3 * BWMAX], F16)
        nc.sync.dma_start(t.rearrange("p (d k w) -> p d k w", d=2, k=3),
                          ap["d01"].transpose([2, 0, 1, 3]))
        return c
    # HWDGE: weights needed in iterations 2-4 (per-dialogue loads share it)
    t = ctile("www", [H, 3 * H], F16)
    nc.sync.dma_start(t.rearrange("p (j h) -> p j h", j=3),
                      ap["www"].transpose([1, 0, 2]))
    t = ctile("w8", [100, 2 * 8 * H], F8)
    nc.sync.dma_start(t.rearrange("p (c r h) -> p c r h", c=2, r=8), ap["w8q"])
    t = ctile("rwl", [100, 4 * H], F16)
    nc.sync.dma_start(t.rearrange("p (c j h) -> p c j h", c=2, j=2), ap["rwl"])
    # SWDGE (Pool queue, idle in prologue): late-need / large constants.
    # tmb is DMA'd in 4 chunks from the main loop so its 6.8us transfer
    # doesn't monopolize the DMA-engine pool during the prologue.
    ctile("tmb", [128, BPC * SEQ], F16)
    t = ctile("brcrow", [1, H], F16)
    nc.gpsimd.dma_start(t[:], ap["brcrow"])
    t = ctile("onesrow", [1, SEQ], F16)
    nc.gpsimd.dma_start(t[:], ap["onesrow"])
    t = ctile("biases", [128, 4], F32)
    nc.gpsimd.dma_start(t[:], ap["biases"])
    t = ctile("wfc", [H, NCLS], F16)
    nc.gpsimd.dma_start(t[:], ap["wfc16"])
    t = ctile("bfcb", [128, BPC * 3 * NCLS], F32)
    nc.gpsimd.dma_start(t[:], ap["bfcb"])
    c.l_out = cpool.tile([128, BPC * 3 * NCLS], F32, name="c_lout")

    # slice helpers
    c.watt_s = lambda ch, u0, uk: c.watt[:, ch * SEQ + u0: ch * SEQ + u0 + uk]
    c.dir_s = lambda dd, k, w: c.d01[:, (dd * 3 + k) * BWMAX: (dd * 3 + k) * BWMAX + w]
    c.win_s = lambda k, w: c.winb[:, k * BWMAX: k * BWMAX + w]
    # DoubleRow lhsT for relation r: [100, 2(ch-planes), H]
    c.w8_dr = lambda r: c.w8.rearrange("p (c r h) -> p c r h", c=2, r=8)[:, :, r, :]
    c.root_s = lambda ch: c.rwl[:, (ch * 2 + 0) * H: (ch * 2 + 0 + 1) * H]
    c.wlind_s = lambda ch: c.rwl[:, (ch * 2 + 1) * H: (ch * 2 + 1 + 1) * H]
    c.w1 = c.www[:, 0 * H:1 * H]
    c.w2 = c.www[:, 1 * H:2 * H]
    c.wlinh = c.www[:, 2 * H:3 * H]
    c.brc = c.biases[:, 0:1]
    c.bgc = c.biases[:, 1:2]
    c.blc = c.biases[:, 2:3]
    return c


def _body(nc, tc, ap, out, pools):
    cpool, io, wk, gpool = pools["cpool"], pools["io"], pools["wk"], pools["gpool"]
    ps_gb, ps_h1, ps_big = pools["ps_gb"], pools["ps_h1"], pools["ps_big"]

    st = {}  # per-dialogue live tiles

    def gb_bank():
        return ps_gb.tile([128, 512], F32, name="gb", tag="gb")

    def loads(b):
        d = st.setdefault(b, {})
        xz = io.tile([128, 2 * 3 * D], F16, name="xz", tag="xz", bufs=9)
        nc.sync.dma_start(xz[:], ap["xz16"][b])
        d["xz"] = d["xn"] = xz  # xn view: cols [0:600) = [k, d] blocks
        d["mc"] = c.mskc[:, b * 6:(b + 1) * 6]

    def xt_s(b, ch, a, w):  # xt [d-chunk, t] slice (cols 600:1200 of xz)
        return st[b]["xz"][:100, 600 + ch * SEQ + a: 600 + ch * SEQ + a + w]

    def s1a(b, c):
        """scale matmuls + one merged exp + win-masked row sums + rm. PE/Act/DVE."""
        d = st[b]
        psc = gb_bank()
        for k, (u0, uk) in enumerate(UT):
            o = SCOFF[k]
            for ch in range(2):
                nc.tensor.matmul(psc[:uk, o:o + BW[k]], c.watt_s(ch, u0, uk),
                                 xt_s(b, ch, C0[k], BW[k]),
                                 start=(ch == 0), stop=(ch == 1))
        p = wk.tile([128, 340], F16, name="p", tag="p")
        nc.scalar.activation(p[:, :], psc[:, :340], AF.Exp)
        d["p"] = p
        d["sm3"] = wk.tile([128, 3], F32, name="sm3", tag="sm3")
        d["rc3"] = wk.tile([128, 3], F32, name="rc3", tag="rc3")

    def s1b(b, c):
        """win-masked row sums -> rm, then Shat'_{s,dd} = (P*mask_s/sums)*dir_dd.
        All DVE, emitted at iteration tail (consumed by G-mms next iter)."""
        d = st[b]
        p = d["p"]
        sm3, rc3 = d["sm3"], d["rc3"]
        d["pw"] = []
        d["rm"] = {}
        for k, (u0, uk) in enumerate(UT):
            pw = wk.tile([128, BWMAX], F16, name=f"pw{k}", tag=f"pw{k}")
            nc.vector.scalar_tensor_tensor(
                pw[:uk, :BW[k]], p[:uk, SCOFF[k]:SCOFF[k] + BW[k]], 1.0,
                c.win_s(k, BW[k])[:uk, :],
                op0=OP.mult, op1=OP.mult,
                accum_out=sm3[:uk, k:k + 1],
            )
            d["pw"].append(pw)
        nc.vector.reciprocal(rc3[:, :], sm3[:, :])
        for k, (u0, uk) in enumerate(UT):
            rm2 = wk.tile([128, 2], F32, name=f"rm{k}", tag=f"rm{k}")
            nc.vector.tensor_scalar_mul(rm2[:uk, :], d["mc"][:uk, k::3],
                                        rc3[:uk, k:k + 1])
            for s in range(2):
                d["rm"][(s, k)] = rm2[:, s:s + 1]
        d["shat"] = {}
        d["xr"] = []
        for k, (u0, uk) in enumerate(UT):
            for dd in range(2):
                stt = wk.tile([128, BWMAX], F16, name=f"st{dd}{k}",
                              tag=f"st{dd}{k}")
                nc.vector.tensor_tensor(
                    stt[:uk, :BW[k]], d["pw"][k][:uk, :BW[k]],
                    c.dir_s(dd, k, BW[k])[:uk, :], op=OP.mult)
                d["shat"][(dd, k)] = stt
            xr = wk.tile([128, 2 * D], F16, name=f"xr{k}", tag=f"xr{k}")
            for s in range(2):
                nc.vector.tensor_scalar_mul(
                    xr[:uk, s * D:(s + 1) * D],
                    d["xn"][:uk, k * D:(k + 1) * D], d["rm"][(s, k)][:uk, :])
            d["xr"].append(xr)

    COMBOS = [(s, dd, ch) for s in range(2) for dd in range(2) for ch in range(2)]

    def s23(bg, bp, c):
        """G banded matmuls + fp8 drains for dialogue bg, interleaved with
        DoubleRow projection + root/bias matmuls for dialogue bp (keeps PE
        fed while Act/DVE chase the PSUM drains)."""
        proj = []  # (tau, idx, s, dd) DoubleRow matmul argument list
        if bp >= 0:
            for tau in range(2):
                for i, (s, dd) in enumerate([(0, 0), (0, 1), (1, 0), (1, 1)]):
                    proj.append((tau, i, s, dd))
        pi = 0
        ph1 = {}
        if bp >= 0:
            for tau in range(2):
                ph1[tau] = ps_h1.tile([128, SEQ], F32, name=f"h1_{tau}",
                                      tag=f"h1_{tau}")
            st[bp]["ph1"] = ph1

        # delay proj until combo 3 so ph1 banks are freed (select/h1f of the
        # older dialogue run at DVE/Act queue fronts) before the first DR mm
        PROJ_SCHED = [0, 0, 0, 2, 2, 2, 1, 1]

        def emit_proj(n):
            nonlocal pi
            for _ in range(n):
                if pi >= len(proj):
                    return
                tau, i, s, dd = proj[pi]
                r = s * 4 + tau * 2 + dd
                g = st[bp]["g"][(s, dd)]
                nc.tensor.matmul(ph1[tau][:, :], c.w8_dr(r),
                                 g.rearrange("p (c t) -> p c t", c=2)[:100, :, :],
                                 start=(i == 0), stop=False,
                                 perf_mode=mybir.MatmulPerfMode.DoubleRow)
                pi += 1
                if pi == len(proj):
                    # root (x256) + bias_r (x256) accumulated into both taus
                    for tau2 in range(2):
                        for ch in range(2):
                            nc.tensor.matmul(ph1[tau2][:, :], c.root_s(ch),
                                             xt_s(bp, ch, 0, SEQ),
                                             start=False, stop=False)
                        nc.tensor.matmul(ph1[tau2][:, :], c.brcrow[:1, :],
                                         c.onesrow[:1, :],
                                         start=False, stop=True)

        if bg >= 0:
            d = st[bg]
            d["g"] = {}
            gi = 0
            for (s, dd, ch) in COMBOS:
                pg = gb_bank()
                for (a, e, ks) in PIECES:
                    for j, k in enumerate(ks):
                        u0, uk = UT[k]
                        nc.tensor.matmul(
                            pg[:100, a:e],
                            d["xr"][k][:uk, s * D + ch * 100: s * D + ch * 100 + 100],
                            d["shat"][(dd, k)][:uk, a - C0[k]:e - C0[k]],
                            start=(j == 0), stop=(j == len(ks) - 1))
                if ch == 0:
                    g = gpool.tile([128, 2 * SEQ], F8, name=f"g{s}{dd}",
                                   tag=f"g{s}{dd}")
                    d["g"][(s, dd)] = g
                else:
                    g = d["g"][(s, dd)]
                if gi not in (1, 3, 5):
                    nc.scalar.activation(g[:100, ch * SEQ:(ch + 1) * SEQ],
                                         pg[:100, :SEQ], AF.Identity, scale=GSCALE)
                else:
                    nc.vector.tensor_scalar_mul(g[:100, ch * SEQ:(ch + 1) * SEQ],
                                                pg[:100, :SEQ], GSCALE)
                emit_proj(PROJ_SCHED[gi])
                gi += 1
        emit_proj(len(proj))

    def s3bv(b, c):
        """target-speaker select + h1f = sel + bias_r + root-part. DVE."""
        d = st[b]
        ph1 = d["ph1"]
        nc.vector.copy_predicated(
            ph1[1][:, :],
            c.tmb.bitcast(I16)[:, b * SEQ:(b + 1) * SEQ],
            ph1[0][:, :])
        h1f = gpool.tile([H, SEQ], F16, name="h1f", tag="h1f")
        nc.scalar.activation(h1f[:], ph1[1][:, :], AF.Identity,
                             scale=1.0 / H1SCALE)
        d["h1f"] = h1f

    def s3bq(b, c):
        """qt = h1f^T W2 (3 u-tiles into one psum) + drain. PE/Act."""
        d = st[b]
        pq = gb_bank()
        for k, (u0, uk) in enumerate(UT):
            nc.tensor.matmul(pq[:uk, k * H:(k + 1) * H],
                             d["h1f"][:, u0:u0 + uk], c.w2,
                             start=True, stop=True)
        qt = gpool.tile([128, 3 * H], F16, name="qt", tag="qt")
        nc.scalar.copy(qt[:], pq[:, :3 * H])
        d["qt"] = qt

    def s3c(b, c):
        """h2 = W1^T h1f + banded qt aggregation + b_gc. PE/Act."""
        d = st[b]
        ph2 = ps_big.tile([H, SEQ], F32, name="big", tag="big")
        nc.tensor.matmul(ph2[:, :], c.w1, d["h1f"][:], start=True, stop=False)
        for (a, e, ks) in PIECES:
            for j, k in enumerate(ks):
                u0, uk = UT[k]
                nc.tensor.matmul(ph2[:, a:e], d["qt"][:uk, k * H:(k + 1) * H],
                                 c.win_s(k, BW[k])[:uk, a - C0[k]:e - C0[k]],
                                 start=False, stop=(j == len(ks) - 1))
        h2 = gpool.tile([H, SEQ], F16, name="h2", tag="h2")
        nc.scalar.activation(h2[:], ph2[:], AF.Identity, bias=c.bgc)
        d["h2"] = h2

    def s3d(b, c):
        """hid = relu(Wlin_d^T x + Wlin_h^T h2 + b_lin). PE/Act."""
        d = st[b]
        phid = ps_big.tile([H, SEQ], F32, name="big", tag="big")
        for ch in range(2):
            nc.tensor.matmul(phid[:, :], c.wlind_s(ch), xt_s(b, ch, 0, SEQ),
                             start=(ch == 0), stop=False)
        nc.tensor.matmul(phid[:, :], c.wlinh, d["h2"][:], start=False, stop=True)
        hid = gpool.tile([H, SEQ], F16, name="hid", tag="hid")
        nc.scalar.activation(hid[:], phid[:], AF.Relu, bias=c.blc)
        d["hid"] = hid

    def s3e(b, c):
        """logits^T per u-tile (n=6) + drain into l_out. PE/DVE."""
        d = st[b]
        plt = gb_bank()
        for k, (u0, uk) in enumerate(UT):
            nc.tensor.matmul(plt[:uk, k * NCLS:(k + 1) * NCLS],
                             d["hid"][:, u0:u0 + uk], c.wfc,
                             start=True, stop=True)
        nc.vector.tensor_copy(c.l_out[:, b * 18:(b + 1) * 18], plt[:, :18])
        del st[b]

    # ---- stage 2 (chunked): bias + log-softmax + output DMA per 4 dialogues
    NCH = 2
    CW = BPC // NCH * 3 * NCLS  # 144 cols per chunk
    s4t = {}

    def s4(cc):
        o = cc * CW
        gn = CW // NCLS  # 24 groups
        lb = s4t.setdefault("lb", cpool.tile([128, BPC * 3 * NCLS], F32,
                                             name="c_lb"))
        nc.vector.tensor_tensor(lb[:, o:o + CW], c.l_out[:, o:o + CW],
                                c.bfcb[:, o:o + CW], op=OP.add)
        l3 = lb[:, o:o + CW].rearrange("p (g c) -> p g c", c=NCLS)
        m9 = s4t.setdefault("m9", cpool.tile([128, BPC * 3], F32, name="c_m9"))
        m96 = m9[:, cc * gn:(cc + 1) * gn]
        nc.vector.reduce_max(m96, l3, axis=mybir.AxisListType.X)
        esb = s4t.setdefault("esb", cpool.tile([128, BPC * 3 * NCLS], F32,
                                               name="c_esb"))
        e3 = esb[:, o:o + CW].rearrange("p (g c) -> p g c", c=NCLS)
        nc.vector.tensor_tensor(e3, l3, m96.broadcast_to([128, gn, NCLS]),
                                op=OP.subtract)
        e2sb = s4t.setdefault("e2sb", cpool.tile([128, BPC * 3 * NCLS], F32,
                                                 name="c_e2sb"))
        nc.scalar.activation(e2sb[:, o:o + CW], esb[:, o:o + CW], AF.Exp)
        s9 = s4t.setdefault("s9", cpool.tile([128, BPC * 3], F32, name="c_s9"))
        s96 = s9[:, cc * gn:(cc + 1) * gn]
        nc.vector.reduce_sum(
            s96, e2sb[:, o:o + CW].rearrange("p (g c) -> p g c", c=NCLS),
            axis=mybir.AxisListType.X)
        lnz = s4t.setdefault("lnz", cpool.tile([128, BPC * 3], F32, name="c_lnz"))
        lnzc = lnz[:, cc * gn:(cc + 1) * gn]
        nc.scalar.activation(lnzc, s96, AF.Ln)
        lsm = s4t.setdefault("lsm", cpool.tile([128, BPC * 3], F32, name="c_lsm"))
        lsmc = lsm[:, cc * gn:(cc + 1) * gn]
        nc.vector.tensor_tensor(lsmc, m96, lnzc, op=OP.add)
        osb = s4t.setdefault("osb", cpool.tile([128, BPC * 3 * NCLS], F32,
                                               name="c_osb"))
        o3 = osb[:, o:o + CW].rearrange("p (g c) -> p g c", c=NCLS)
        nc.vector.tensor_tensor(o3, l3, lsmc.broadcast_to([128, gn, NCLS]),
                                op=OP.subtract)
        nc.sync.dma_start(out[:, o:o + CW], osb[:, o:o + CW])

    # ---- prologue + pipelined loop ----
    c = _load_consts(nc, cpool, ap, phase=0)
    loads(0)
    _load_consts(nc, cpool, ap, c=c, phase=1)
    loads(1)
    loads(2)
    _load_consts(nc, cpool, ap, c=c, phase=2)

    BPN = BPC // NCH
    TQ = BPC * SEQ // 8
    for i in range(BPC + 8):
        if 1 <= i < 9:
            j = i - 1
            nc.gpsimd.dma_start(c.tmb[:, j * TQ:(j + 1) * TQ],
                                ap["tmb16"][:, j * TQ:(j + 1) * TQ])
        if 3 <= i + 1 < BPC:
            loads(i + 1)
        if i < BPC:
            s1a(i, c)  # scale mms (PE front) + exp (Act front)
        if 0 <= i - 3 < BPC:
            s3bv(i - 3, c)  # select + h1f at DVE/Act queue fronts
        # G drains hit DVE/Act queues right after so the PSUM ring turns
        s23(i - 1 if i - 1 < BPC else -1, i - 2 if i - 2 < BPC else -1, c)
        if 0 <= i - 3 < BPC:
            s3bq(i - 3, c)
        if 0 <= i - 5 < BPC:
            s3c(i - 5, c)
        if 0 <= i - 6 < BPC:
            s3d(i - 6, c)
        if 0 <= i - 7 < BPC:
            s3e(i - 7, c)
        if (i - 8) % BPN == BPN - 1 and 0 <= i - 8 < BPC:
            s4((i - 8) // BPN)
        if i < BPC:
            s1b(i, c)  # win-stt/rm/dir-stts at DVE queue tail


def _host_prep(inputs):
    feats = np.asarray(inputs["features"], dtype=np.float32)    # (300,256,200)
    spk = np.asarray(inputs["speakers"])                        # (300,256)
    W_att = np.asarray(inputs["W_att"], dtype=np.float32)
    basis = np.asarray(inputs["basis"], dtype=np.float32)
    comp = np.asarray(inputs["comp"], dtype=np.float32)
    root = np.asarray(inputs["root"], dtype=np.float32)
    bias_r = np.asarray(inputs["bias_r"], dtype=np.float32)
    W1 = np.asarray(inputs["W1"], dtype=np.float32)
    W2 = np.asarray(inputs["W2"], dtype=np.float32)
    b_gc = np.asarray(inputs["b_gc"], dtype=np.float32)
    W_lin = np.asarray(inputs["W_lin"], dtype=np.float32)
    b_lin = np.asarray(inputs["b_lin"], dtype=np.float32)
    W_fc = np.asarray(inputs["W_fc"], dtype=np.float32)
    b_fc = np.asarray(inputs["b_fc"], dtype=np.float32)

    i = np.arange(SEQ)[:, None]
    j = np.arange(SEQ)[None, :]
    win = (j >= i - WP) & (j <= i + WF)
    dir0 = (win & (i < j)).astype(np.float32)
    dir1 = (win & (i >= j)).astype(np.float32)
    winm = win.astype(np.float32)

    # band-local masks, padded to (3, 128, BWMAX)
    def bandify(m):
        o = np.zeros((3, 128, BWMAX), np.float16)
        for k, (u0, uk) in enumerate(UT):
            o[k, :uk, :BW[k]] = m[u0:u0 + uk, C0[k]:C1[k]]
        return o

    d01 = np.stack([bandify(dir0), bandify(dir1)])              # (2,3,128,148)
    winb = bandify(winm)                                        # (3,128,148)

    import ml_dtypes
    w8 = np.einsum("rb,bdh->rdh", comp, basis).astype(np.float32)  # (8,200,128)
    w8q = np.ascontiguousarray(
        (w8 * W8SCALE).reshape(8, 2, 100, H).transpose(2, 1, 0, 3)
    ).astype(ml_dtypes.float8_e4m3fn)                           # (100,2,8,128)
    watt16 = np.ascontiguousarray(
        W_att.reshape(2, 100, SEQ).transpose(1, 0, 2)).astype(np.float16)
    rwl = np.stack([(root * H1SCALE).reshape(2, 100, H),
                    np.ascontiguousarray(W_lin[:D]).reshape(2, 100, H)],
                   axis=2).transpose(1, 0, 2, 3)                # (100,2,2,128)
    rwl = np.ascontiguousarray(rwl).astype(np.float16)
    brcrow = (bias_r * H1SCALE).astype(np.float16).reshape(1, H)
    onesrow = np.ones((1, SEQ), np.float16)
    www = np.stack([W1, W2, np.ascontiguousarray(W_lin[D:])]).astype(np.float16)
    biases = np.zeros((128, 4), np.float32)
    biases[:, 0] = bias_r
    biases[:, 1] = b_gc
    biases[:, 2] = b_lin
    bfcb = np.broadcast_to(np.tile(b_fc, BPC * 3)[None, :],
                           (128, BPC * 3 * NCLS)).astype(np.float32)
    bfcb = np.ascontiguousarray(bfcb)

    shared = {
        "d01": d01, "winb": winb,
        "watt16": watt16, "w8q": w8q, "brcrow": brcrow, "onesrow": onesrow,
        "rwl": rwl, "www": www,
        "wfc16": W_fc.astype(np.float16),
        "biases": biases, "bfcb": bfcb,
    }

    in_maps = []
    for cid in range(NCORES):
        bs = slice(cid * BPC, (cid + 1) * BPC)
        fb = feats[:, bs, :]                                    # (300,32,200)
        xz = np.zeros((BPC, 128, 2 * 3 * D), np.float16)
        xn = fb.transpose(1, 0, 2)                              # (32,300,200)
        for k, (u0, uk) in enumerate(UT):
            xz[:, :uk, k * D:(k + 1) * D] = xn[:, u0:u0 + uk, :]
        xt = fb.transpose(1, 2, 0)                              # (32,200,300)
        xz[:, :100, 600:900] = xt[:, :100, :]
        xz[:, :100, 900:1200] = xt[:, 100:, :]
        sp = spk[:, bs].T                                       # (32,300)
        mskc = np.zeros((128, BPC, 6), np.float32)
        for s in range(2):
            for k, (u0, uk) in enumerate(UT):
                mskc[:uk, :, s * 3 + k] = (sp[:, u0:u0 + uk] == s).T
        tmb16 = np.broadcast_to(
            (sp == 0).astype(np.float16).reshape(1, BPC * SEQ),
            (128, BPC * SEQ))
        m = {"xz16": xz, "mskc": np.ascontiguousarray(mskc.reshape(128, BPC * 6)),
             "tmb16": np.ascontiguousarray(tmb16)}
        m.update(shared)
        in_maps.append(m)
    return in_maps


def get_program():
    if "nc" not in _CACHE:
        _CACHE["nc"] = _build_program()
    return _CACHE["nc"]


def kernel(**inputs):
    nc = get_program()
    in_maps = _host_prep(inputs)
    res = bass_utils.run_bass_kernel_spmd(nc, in_maps, core_ids=list(range(NCORES)))
    outs = []
    for c in range(NCORES):
        arr = np.asarray(res.results[c]["out"])            # (128, 576)
        arr = arr.reshape(128, BPC, 3, NCLS).transpose(1, 2, 0, 3)
        outs.append(arr.reshape(BPC, 384, NCLS)[:, :SEQ, :].reshape(-1, NCLS))
    return np.concatenate(outs, axis=0)
